# revision 45
# baseline (speedup 1.0000x reference)
"""BigBird block-sparse attention for Trainium2, 8-core SPMD.

Sharding: head-parallel. Each core owns 2 of the 16 heads (both batches).
  - q/k/v projections computed only for the core's 128 feature slice
    (full hidden_states replicated, weights sliced column-wise).
  - attention fully local per (batch, head).
  - out_proj tensor-parallel on the head (contraction) dim: each core
    emits a full-shape partial; the host sums the 8 partials and adds
    the output bias.

Performance model notes (measured on HW): the PE is INSTRUCTION-ISSUE
bound at ~110ns per matmul when matmuls are narrow — wide matmuls are
essential.  The attention schedule therefore uses UNION runs: for each
128-key tile, one full-height score matmul per run of consecutive
query blocks attending EITHER half (64-key block) of the tile.  Query
blocks attending only one half get the other half's exp values zeroed
in SBUF (cheap DVE/Pool memsets) so that a single K=128 AV matmul per
run is correct — the appended ones column then also yields the correct
softmax denominator Z.

On-device layout choices:
  - activations feature-major (features on partitions, tokens on free dim)
  - scores computed transposed: S_T[key, query] = k_j^T q, so that
    * AV is a natural matmul (contraction = keys = partitions),
    * the softmax denominator Z falls out of a ones-column appended to V^T,
    * normalization folds into the PSUM->SBUF context copy as a
      partition-broadcast multiply by 1/Z.
  - softmax skips max-subtraction (scores are O(1) after the 1/8 scale;
    exp cannot overflow fp32 for this distribution; softmax is shift
    invariant so the reference is matched).
  - v is projected feature-major on all 128 partitions (both heads) and
    transposed per 128-token tile with ONE PE transpose, then copied to
    the [v | 1] slots (vaug2) both heads at once.
  - out_proj for batch 0 is interleaved into the attention phase so the
    finalize chain of the last pair is hidden behind real PE work.
"""

import numpy as np
import ml_dtypes
from contextlib import ExitStack

# ----- problem constants (hardcoded per contract) --------------------------
EMBED_DIM = 1024
NUM_HEADS = 16
HEAD_DIM = 64           # d per head
WINDOW = 3
N_RAND = 3
BLOCK = 64
BATCH = 2
SEQ = 2048
NB = SEQ // BLOCK       # 32 key/query blocks per sequence
N_CORES = 8
HPC = NUM_HEADS // N_CORES      # heads per core = 2
FPC = HPC * HEAD_DIM            # feature slice per core = 128
T = BATCH * SEQ                 # 4096 tokens
NKT = NB // 2                   # 16 key tiles of 128 keys per (b,h)
SCALE = HEAD_DIM ** -0.5

BF16 = ml_dtypes.bfloat16
FP8 = ml_dtypes.float8_e4m3fn

# score-chunk window width in psum columns (2 PSUM banks; 3-deep ring so
# the PE can run chunks ahead of the exp stream)
CHUNK_W = 1024
PSUM_BANK = 512  # fp32 elements per bank


def _block_attend() -> np.ndarray:
    """attend[r, kb]: query block r attends key block kb.

    Block-granular replica of the reference _bigbird_mask (the mask is
    block-constant: global first block rows/cols, +-WINDOW band, and
    N_RAND random blocks per row drawn with RandomState(0))."""
    att = np.zeros((NB, NB), dtype=bool)
    att[0, :] = True
    att[:, 0] = True
    blk = np.arange(NB)
    att |= np.abs(blk[:, None] - blk[None, :]) <= WINDOW
    rng = np.random.RandomState(0)
    for b in range(1, NB):
        avail = [x for x in range(1, NB) if abs(x - b) > WINDOW]
        if avail:
            sel = rng.choice(avail, size=min(N_RAND, len(avail)), replace=False)
            att[b, sel] = True
    return att


def _runs_of(mask_1d: np.ndarray):
    """[(r0, nblocks)] maximal runs of consecutive True entries."""
    runs = []
    for r in np.flatnonzero(mask_1d):
        if runs and runs[-1][0] + runs[-1][1] == r:
            runs[-1][1] += 1
        else:
            runs.append([int(r), 1])
    return [(r0, n) for r0, n in runs]


def build_schedule(qb_lo=0, qb_hi=NB):
    """Union-run schedule, packed into CHUNK_W-wide psum windows.

    Returns list of chunks; each chunk is a dict:
      W     : used width in psum columns
      runs  : [(j, r0, nblk, off)] score/AV runs — query blocks
              r0..r0+nblk attend at least one half of key tile j; the
              score matmul is full-height (128 keys), AV is one K=128
              matmul per run against the [v|1] slot of tile j.
      zeros : [(row0, nrows, off, w)] exp outputs to zero in E (the
              unattended half of single-side columns).
    """
    att = _block_attend()
    chunks = []
    cur = dict(W=0, runs=[], zeros=[])
    for j in range(NKT):
        a0, a1 = att[:, 2 * j], att[:, 2 * j + 1]
        u = (a0 | a1).copy()
        # merge runs separated by exactly 1 block (measured optimum:
        # per-matmul overhead beats the extra 64 exp/AV columns at gap 1
        # but not at gap 2); gap columns get fully zeroed in E (harmless
        # for AV and Z)
        u[:qb_lo] = False
        u[qb_hi:] = False
        g0 = None
        for r0, n in _runs_of(u):
            if g0 is not None and r0 - g0 <= 1:
                u[g0:r0] = True
            g0 = r0 + n
        for r0, n in _runs_of(u):
            while n > 0:
                space = (CHUNK_W - cur["W"]) // 64
                if space == 0:
                    chunks.append(cur)
                    cur = dict(W=0, runs=[], zeros=[])
                    continue
                take = min(n, space)
                off = cur["W"]
                cur["runs"].append((j, r0, take, off))
                # zero rects for single-side query blocks, merged along
                # consecutive same-type blocks
                qq = r0
                while qq < r0 + take:
                    def typ(q):
                        if a0[q] and a1[q]:
                            return 2
                        if a0[q]:
                            return 0
                        if a1[q]:
                            return 1
                        return 3  # merged gap: zero all 128 rows
                    t = typ(qq)
                    q2 = qq
                    while q2 < r0 + take and typ(q2) == t:
                        q2 += 1
                    if t == 3:
                        cur["zeros"].append((0, 128,
                                             off + 64 * (qq - r0),
                                             64 * (q2 - qq)))
                    elif t != 2:
                        # even-only (t=0): zero odd rows 64:128;
                        # odd-only (t=1): zero even rows 0:64
                        cur["zeros"].append((64 * (1 - t), 64,
                                             off + 64 * (qq - r0),
                                             64 * (q2 - qq)))
                    qq = q2
                cur["W"] += 64 * take
                r0 += take
                n -= take
    if cur["runs"]:
        chunks.append(cur)
    for c in chunks:
        zs = sorted(c["zeros"])
        merged = []
        for z in zs:
            if merged and merged[-1][0] == z[0] and \
                    merged[-1][1] == z[1] and \
                    merged[-1][2] + merged[-1][3] == z[2]:
                merged[-1] = (merged[-1][0], merged[-1][1], merged[-1][2],
                              merged[-1][3] + z[3])
            else:
                merged.append(z)
        c["zeros"] = merged
    return chunks


def _bank_split(off, w, bank=PSUM_BANK):
    """split [off, off+w) at bank boundaries -> [(off, w), ...]"""
    out = []
    while w > 0:
        room = bank - (off % bank)
        take = min(room, w)
        out.append((off, take))
        off += take
        w -= take
    return out


# ---------------------------------------------------------------------------
# numpy golden of the exact on-device algorithm (fp32, validates schedule)
# ---------------------------------------------------------------------------
def numpy_golden(hidden_states, wq, bq, wk, bk, wv, bv, wo, bo):
    hs = np.asarray(hidden_states, np.float32).reshape(T, EMBED_DIM)
    chunks = (build_schedule(0, NB // 2) + build_schedule(NB // 2, NB))
    out = np.zeros((T, EMBED_DIM), np.float32)
    for c in range(N_CORES):
        f = slice(FPC * c, FPC * (c + 1))
        q = hs @ np.asarray(wq, np.float32)[f, :].T  # (T, 128)
        k = hs @ np.asarray(wk, np.float32)[f, :].T
        v = hs @ np.asarray(wv, np.float32)[f, :].T
        ctx_all = np.zeros((FPC, T), np.float32)
        for b in range(BATCH):
            for hl in range(HPC):
                d = slice(64 * hl, 64 * hl + 64)
                tok = slice(b * SEQ, (b + 1) * SEQ)
                qb = q[tok, d]   # (2048, 64)
                kb = k[tok, d]
                vb = v[tok, d]
                v_aug = np.concatenate(
                    [vb, np.ones((SEQ, 64), np.float32)], 1)
                ctx = np.zeros((128, SEQ), np.float32)
                for ch in chunks:
                    E = np.zeros((128, ch["W"]), np.float32)
                    for j, r0, nblk, off in ch["runs"]:
                        kk = slice(j * 128, j * 128 + 128)
                        qq = slice(64 * r0, 64 * (r0 + nblk))
                        s = kb[kk, :] @ qb[qq, :].T  # (128 keys, queries)
                        E[:, off:off + 64 * nblk] = np.exp(SCALE * s)
                    for row0, nrows, off, w in ch["zeros"]:
                        E[row0:row0 + nrows, off:off + w] = 0.0
                    for j, r0, nblk, off in ch["runs"]:
                        kk = slice(j * 128, j * 128 + 128)
                        qq = slice(64 * r0, 64 * (r0 + nblk))
                        ctx[:, qq] += v_aug[kk, :].T @ E[:, off:off + 64 * nblk]
                ctx_n = ctx[:64, :] / ctx[64:65, :]
                ctx_all[d, tok] = ctx_n
        partial = np.asarray(wo, np.float32)[:, f] @ ctx_all  # (1024, T)
        out += partial.T
    out = out + np.asarray(bo, np.float32)
    return out.reshape(BATCH, SEQ, EMBED_DIM)


# ---------------------------------------------------------------------------
# Bass/Tile kernel (one core's program; SPMD across 8 cores)
# ---------------------------------------------------------------------------
def _trace_core_program():
    import concourse.bass as bass
    import concourse.mybir as mybir
    import concourse.tile as tile
    from concourse import bacc

    dt = mybir.dt
    chunks_h = [build_schedule(0, NB // 2), build_schedule(NB // 2, NB)]

    nc = bacc.Bacc(None, target_bir_lowering=False)
    with tile.TileContext(nc) as tc:
        with ExitStack() as top:
            dram = top.enter_context(tc.tile_pool(name="dram", bufs=1, space="DRAM"))
            hT_d = dram.tile([EMBED_DIM, T], dt.bfloat16, kind="ExternalInput",
                             name="hT", uniquify=False)
            wqkvT_d = dram.tile([EMBED_DIM, 3 * FPC], dt.bfloat16,
                                kind="ExternalInput", name="wqkvT",
                                uniquify=False)
            woT_d = dram.tile([FPC, EMBED_DIM], dt.bfloat16,
                              kind="ExternalInput", name="woT", uniquify=False)
            ident_d = dram.tile([128, 128], dt.bfloat16,
                                kind="ExternalInput", name="ident",
                                uniquify=False)
            out_d = dram.tile([EMBED_DIM, T], dt.bfloat16,
                              kind="ExternalOutput", name="out", uniquify=False)

            # ---- persistent SBUF tensors -----------------------------------
            persist = top.enter_context(tc.tile_pool(name="persist", bufs=1))
            # NOTE: fp8e4m3 projection operands with DoubleRow matmuls were
            # tried (works, ~10us faster) but the quantization error of a
            # random-sign dot product does NOT average down with contraction
            # length — measured 6.5%% relative error, over the 2%% budget.
            wqkv = persist.tile([128, 8, 3 * FPC], dt.bfloat16, name="wqkv_sb")
            woT = persist.tile([128, EMBED_DIM], dt.bfloat16, name="wo_sb")
            ident = persist.tile([128, 128], dt.bfloat16, name="ident_sb")
            # q/k head-major on 64 partitions (base-0 only: matmuls with
            # base-partition-64 contraction operands hit a codegen/HW bug)
            q_sb = persist.tile([64, HPC * T], dt.bfloat16, name="q_sb")
            k_sb = persist.tile([64, HPC * T], dt.bfloat16, name="k_sb")
            # v feature-major, both heads on 128 partitions (transposed
            # per-tile on the PE)
            vfm = persist.tile([128, T], dt.bfloat16, name="vfm_sb")
            # per (b,hl): [v | 1...1] slots per 128-key tile, K=128 for
            # all AV.  Slot cols 64:128 are ALL ones: every AV matmul then
            # emits Z replicated across output rows 64:128 — the partition
            # broadcast of the softmax denominator comes free on the PE
            # (matmul cost depends only on the moving columns).
            vaug2 = persist.tile([128, BATCH * HPC, NKT * 128], dt.bfloat16,
                                 name="vaug2_sb")
            # normalized context, one tile per batch so out-proj for batch 0
            # doesn't pick up a false dependency on batch-1 writers
            ctx_b = [persist.tile([128, SEQ], dt.bfloat16, name=f"ctx_sb{b}")
                     for b in range(BATCH)]
            zt = persist.tile([128, 256], dt.bfloat16, name="zt_sb")

            # E/finalize pools span phase 1 and attention (the first
            # head's first-half exp output is produced during phase 1)
            ep = top.enter_context(tc.tile_pool(name="e_pool", bufs=1))
            fp = top.enter_context(tc.tile_pool(name="fin_pool", bufs=3))

            # weight DMAs first, one per contraction chunk (finer deps:
            # the e=0 projection matmul starts as soon as slice 0 lands)
            wq_view = wqkvT_d.rearrange("(e p) f -> p e f", p=128)
            for e in range(8):
                (nc.gpsimd if e % 2 else nc.scalar).dma_start(
                    out=wqkv[:, e, :], in_=wq_view[:, e, :])

            HSEQ = SEQ // 2
            E_store = {}

            def emit_half_scores(b, hl, half, scp, ep, tag):
                qtok0 = hl * T + b * SEQ  # column base in q/k (head-major)
                E_tiles = []
                zi = 0
                for ci, ch in enumerate(chunks_h[half]):
                    W = ch["W"]
                    S = scp.tile([128, CHUNK_W], dt.float32, tag="S")
                    E = ep.tile([128, W], dt.bfloat16,
                                tag=f"E{tag}{half}{ci}",
                                name=f"E{tag}{half}{ci}{b}{hl}")
                    E_tiles.append(E)
                    for j, r0, nblk, off in ch["runs"]:
                        kcol0 = qtok0 + 128 * j
                        for o, w in _bank_split(off, 64 * nblk):
                            qc = qtok0 + 64 * r0 + (o - off)
                            nc.tensor.matmul(
                                S[:, o:o + w],
                                k_sb[:, kcol0:kcol0 + 128],
                                q_sb[:, qc:qc + w],
                                start=True, stop=True)
                    nc.scalar.activation(
                        E[:, :W], S[:, :W],
                        mybir.ActivationFunctionType.Exp, scale=SCALE)
                    # zero the unattended half of single-side columns
                    for row0, nrows, off, w in ch["zeros"]:
                        (nc.vector if zi % 2 else nc.gpsimd).memset(
                            E[row0:row0 + nrows, off:off + w], 0.0)
                        zi += 1
                E_store[(b, hl, half)] = E_tiles

            def emit_half_avfin(b, hl, half, ctxp, fp):
                p = b * HPC + hl
                qoff = half * HSEQ  # query-column base of this half
                ctx = ctxp.tile([128, HSEQ], dt.float32, tag="ctx")
                # PSUM start=True arms the whole bank for lazy zeroing:
                # issue it exactly once per ctx bank (the j=0 runs cover
                # every query column, so all banks start early).
                ctx_bank_started = [False] * (HSEQ // PSUM_BANK)
                E_tiles = E_store[(b, hl, half)]
                # AV accumulate (+Z via ones half), K=128 always
                for ci, ch in enumerate(chunks_h[half]):
                    E = E_tiles[ci]
                    for j, r0, nblk, off in ch["runs"]:
                        lhsT = vaug2[:, p, 128 * j:128 * j + 128]
                        for o, w in _bank_split(64 * r0, 64 * nblk):
                            eo = off + (o - 64 * r0)
                            co = o - qoff
                            bank = co // PSUM_BANK
                            st = not ctx_bank_started[bank]
                            ctx_bank_started[bank] = True
                            nc.tensor.matmul(
                                ctx[:, co:co + w], lhsT,
                                E[:, eo:eo + w],
                                start=st, stop=False,
                                skip_group_check=True)
                # finalize: ctx rows 64:128 already hold Z on every row
                # (ones half of the vaug2 slots), so 1/Z is a straight
                # 64-partition reciprocal via one DVE instruction.
                zsb = fp.tile([64, HSEQ], dt.float32, tag="zsb")
                nc.vector.tensor_copy(zsb[:], ctx[64:128, :])
                rbc = fp.tile([64, HSEQ], dt.float32, tag="rbc")
                nc.vector.reciprocal_approx_fast(rbc[:], zsb[:])
                for cc in range(HSEQ // 512):
                    csl = slice(512 * cc, 512 * (cc + 1))
                    nc.vector.tensor_tensor(
                        out=ctx_b[b][64 * hl:64 * hl + 64,
                                     qoff + 512 * cc:
                                     qoff + 512 * cc + 512],
                        in0=ctx[0:64, csl],
                        in1=rbc[:, csl],
                        op=mybir.AluOpType.mult)

            def emit_pair(b, hl, scp, ctxp, ep, fp, tag):
                for half in range(2):
                    emit_half_scores(b, hl, half, scp, ep, tag)
                    emit_half_avfin(b, hl, half, ctxp, fp)


            NCHUNK = T // 512
            hT_pool = tc.tile_pool(name="hT_pool", bufs=1)
            with hT_pool as hp:
                hT = hp.tile([128, 8, T], dt.bfloat16, name="hT_sb")

                # hT DMA dispatches spread over 3 engine queues (descriptor
                # generation is ~600ns of engine time apiece); all issued
                # before any memset/copy work so the queues drain into DMA
                # as early as possible
                def h_dma(n, e):
                    eng = (nc.sync if e < 4 else
                           nc.scalar if e < 6 else nc.gpsimd)
                    eng.dma_start(
                        out=hT[:, e, 512 * n:512 * n + 512],
                        in_=hT_d[128 * e:128 * e + 128, 512 * n:512 * n + 512])

                for e in range(8):
                    h_dma(0, e)
                # wo/ident aren't needed until attention — dispatch after
                # the first hidden-state chunk so they don't delay it
                nc.gpsimd.dma_start(out=woT[:], in_=woT_d[:])
                nc.gpsimd.dma_start(out=ident[:], in_=ident_d[:])

                # ~3us of dummy matmuls while DMAs stream: keeps the PE
                # pipeline warm so the projection starts at full p-state,
                # and writes every cell of all 8 PSUM banks once (launders
                # boot-garbage psum).  Closed before proj pools open.
                nc.vector.memset(zt[:], 0.0)
                with tc.tile_pool(name="warm_ps", bufs=8, space="PSUM") as wps:
                    for i in range(10):
                        w = wps.tile([128, 512], dt.float32, tag="warm")
                        nc.tensor.matmul(w[:, 0:256], zt[:, 0:128],
                                         zt[:, 0:256], start=True, stop=True,
                                         skip_group_check=True)

                for n in range(1, NCHUNK):
                    for e in range(8):
                        h_dma(n, e)

                # vaug2 ones-half preset (strided memsets on DVE)
                for p in range(BATCH * HPC):
                    s2 = vaug2[:, p, :].rearrange("p (m c) -> p m c", c=128)
                    nc.vector.memset(s2[:, :, 64:128], 1.0)

                # ---- phase 1: q/k/v projections + per-tile v transpose -----
                # psum->sbuf copies: only DVE and ACT can read PSUM.
                # Projection copies go on DVE alone — the ACT queue holds
                # ~15us of hT DMA dispatches at this point and would stall
                # the proj psum ring.  The vaug2 slot copies (not needed
                # until attention) alternate DVE/ACT.
                cp = nc.vector.tensor_copy
                vt_rot = [nc.vector.tensor_copy, nc.scalar.copy]
                vt_i = [0]

                def vt_cp(dst, src):
                    vt_rot[vt_i[0] % 2](dst, src)
                    vt_i[0] += 1

                with tc.tile_pool(name="proj_ps", bufs=1, space="PSUM") as pps, \
                        tc.tile_pool(name="vt_ps", bufs=1, space="PSUM") as vtp, \
                        tc.tile_pool(name="sc_pre", bufs=2,
                                     space="PSUM") as scpA:
                    for n in range(NCHUNK):
                        # after batch-0's tokens (chunks 0-3) are projected,
                        # emit the first head's first-half scores+exp: the
                        # ACT engine chews its ~10us of exp work while the
                        # PE projects batch 1 — attention then starts one
                        # half-pair ahead
                        if n == 4:
                            emit_half_scores(0, 0, 0, scpA, ep, "a")
                        tsl = slice(512 * n, 512 * (n + 1))
                        # q and k (feature-major, head-split into q_sb/k_sb)
                        for tg, wsl, dst in [("pq", slice(0, 128), q_sb),
                                             ("pk", slice(128, 256), k_sb)]:
                            ps = pps.tile([128, 512], dt.float32, tag=tg)
                            for e in range(8):
                                nc.tensor.matmul(ps[:], wqkv[:, e, wsl],
                                                 hT[:, e, tsl],
                                                 start=(e == 0), stop=(e == 7))
                            for hl in range(HPC):
                                cp(dst[:, hl * T + 512 * n:
                                       hl * T + 512 * n + 512],
                                   ps[64 * hl:64 * hl + 64, :])
                        # v: both heads on 128 partitions, single wide copy
                        ps = pps.tile([128, 512], dt.float32, tag="pv")
                        for e in range(8):
                            nc.tensor.matmul(ps[:], wqkv[:, e, 256:384],
                                             hT[:, e, tsl],
                                             start=(e == 0), stop=(e == 7))
                        cp(vfm[:, tsl], ps[:])
                        # transpose the 4 completed 128-token tiles of the
                        # PREVIOUS chunk (vfm write must land first; lag one
                        # chunk so the copy has completed)
                        for gg in range(4):
                            g = 4 * (n - 1) + gg if n > 0 else None
                            if g is None:
                                break
                            b, jj = g // NKT, g % NKT
                            vt = vtp.tile([128, 128], dt.bfloat16, tag="vt")
                            nc.tensor.transpose(
                                vt[:], vfm[:, 128 * g:128 * g + 128],
                                ident[:])
                            p0 = b * HPC
                            vt_cp(vaug2[:, p0:p0 + 2, 128 * jj:128 * jj + 64],
                                  vt[:].rearrange("p (h c) -> p h c", h=2))
                    # last chunk's 4 tiles
                    for gg in range(4):
                        g = 4 * (NCHUNK - 1) + gg
                        b, jj = g // NKT, g % NKT
                        vt = vtp.tile([128, 128], dt.bfloat16, tag="vt")
                        nc.tensor.transpose(
                            vt[:], vfm[:, 128 * g:128 * g + 128], ident[:])
                        p0 = b * HPC
                        vt_cp(vaug2[:, p0:p0 + 2, 128 * jj:128 * jj + 64],
                              vt[:].rearrange("p (h c) -> p h c", h=2))

            # ---- attention + interleaved out-projection --------------------
            def emit_oproj(opp, opsb, eo_list, b, engines):
                # 2-bank psum tiles: 2 matmuls share one wide copy (fewer
                # psum->sbuf instructions; only DVE/ACT can read PSUM)
                ei = 0
                for eo in eo_list:
                    ob = opsb.tile([128, SEQ], dt.bfloat16, tag="ob")
                    for nn in range(SEQ // 1024):
                        ps = opp.tile([128, 1024], dt.float32, tag="op")
                        for half in range(2):
                            csl = slice(1024 * nn + 512 * half,
                                        1024 * nn + 512 * half + 512)
                            nc.tensor.matmul(
                                ps[:, 512 * half:512 * half + 512],
                                woT[:, 128 * eo:128 * eo + 128],
                                ctx_b[b][:, csl], start=True, stop=True)
                        engines[ei % len(engines)](
                            ob[:, 1024 * nn:1024 * nn + 1024], ps[:])
                        ei += 1
                    (nc.sync if eo % 2 else nc.gpsimd).dma_start(
                        out=out_d[128 * eo:128 * eo + 128,
                                  b * SEQ:(b + 1) * SEQ],
                        in_=ob[:])

            # segment 1: pairs (0,0),(0,1),(1,0) — scp before ctxp so the
            # out-proj pool that follows lands on the score banks (whose
            # last readers finished long ago), not the ctx banks
            with tc.tile_pool(name="sc_ps", bufs=3, space="PSUM") as scp, \
                    tc.tile_pool(name="ctx_ps", bufs=1, space="PSUM") as ctxp:
                emit_half_avfin(0, 0, 0, ctxp, fp)
                emit_half_scores(0, 0, 1, scp, ep, "a")
                emit_half_avfin(0, 0, 1, ctxp, fp)
                emit_pair(0, 1, scp, ctxp, ep, fp, "a")
                emit_pair(1, 0, scp, ctxp, ep, fp, "a")

            # out-proj batch 0, first half of eo (runs while pair (1,0)'s
            # finalize chain completes; copies on ACT so the DVE-resident
            # finalize chain doesn't block them)
            with tc.tile_pool(name="op_ps1", bufs=2, space="PSUM") as opp, \
                    tc.tile_pool(name="op_sb1", bufs=2) as opsb:
                emit_oproj(opp, opsb, range(0, 4), 0,
                           [nc.vector.tensor_copy, nc.scalar.copy,
                            nc.vector.tensor_copy, nc.vector.tensor_copy])

            # segment 2: last pair (1,1)
            with tc.tile_pool(name="sc_ps2", bufs=3, space="PSUM") as scp, \
                    tc.tile_pool(name="ctx_ps2", bufs=1, space="PSUM") as ctxp:
                emit_pair(1, 1, scp, ctxp, ep, fp, "a")

            # out-proj: rest of batch 0 (covers the last finalize chain,
            # ACT copies), then batch 1 (alternating engines)
            with tc.tile_pool(name="op_ps2", bufs=3, space="PSUM") as opp, \
                    tc.tile_pool(name="op_sb2", bufs=3) as opsb:
                emit_oproj(opp, opsb, range(4, 8), 0, [nc.scalar.copy])
                emit_oproj(opp, opsb, range(0, 8), 1,
                           [nc.scalar.copy, nc.vector.tensor_copy])

    nc.compile()
    _dedup_ldweights(nc)
    return nc


def _dedup_ldweights(nc):
    """Remove PE weight reloads of the already-loaded stationary tile.

    bass pairs every matmul with an explicit InstLdweights; on HW the
    load serializes ~130ns of PE time apiece.  Consecutive matmuls here
    frequently share the stationary operand (score runs per key tile,
    AV bank splits per [v|1] slot, out-proj column tiles per eo slice),
    so the repeat loads are pure waste.  Only waitless/updateless loads
    are removed, which leaves every semaphore count intact."""
    removed = 0
    for blk in nc.main_func.blocks:
        insts = blk.instructions
        last_key = None
        to_remove = []
        for i in insts:
            eng = getattr(i, "engine", None)
            if eng is None or str(eng) != "EngineType.PE":
                continue
            tn = type(i).__name__
            if tn == "InstLdweights":
                key = (str(i.ins[0]), str(getattr(i, "perf_mode", None)),
                       str(getattr(i, "is_transpose", None)),
                       str(getattr(i, "tile_position", None)))
                si = i.sync_info
                clean = si is None or (len(si.on_wait) == 0
                                       and len(si.on_update) == 0)
                if clean and key == last_key:
                    to_remove.append(i)
                else:
                    last_key = key
            elif tn in ("InstMatmult", "InstEventSemaphore"):
                pass  # neither clobbers the loaded weights
            else:
                last_key = None
        for i in to_remove:
            insts.remove(i)
        removed += len(to_remove)


_NC_CACHE = None


def make_in_maps(hs, wq, wk, wv, wo):
    hT = np.ascontiguousarray(
        np.asarray(hs, np.float32).reshape(T, EMBED_DIM).T).astype(BF16)
    ident = np.eye(128, dtype=np.float32).astype(BF16)
    wq = np.asarray(wq, np.float32)
    wk = np.asarray(wk, np.float32)
    wv = np.asarray(wv, np.float32)
    wo = np.asarray(wo, np.float32)
    in_maps = []
    for c in range(N_CORES):
        f = slice(FPC * c, FPC * (c + 1))
        wqkvT = np.concatenate([wq[f, :].T, wk[f, :].T, wv[f, :].T], axis=1)
        in_maps.append({
            "hT": hT,
            "wqkvT": np.ascontiguousarray(wqkvT).astype(BF16),
            "woT": np.ascontiguousarray(wo[:, f].T).astype(BF16),
            "ident": ident,
        })
    return in_maps


def kernel(hidden_states, wq, bq, wk, bk, wv, bv, wo, bo):
    global _NC_CACHE
    hs = np.asarray(hidden_states, np.float32)
    wq = np.asarray(wq, np.float32)
    wk = np.asarray(wk, np.float32)
    wv = np.asarray(wv, np.float32)
    wo = np.asarray(wo, np.float32)
    bq = np.asarray(bq, np.float32)
    bk = np.asarray(bk, np.float32)
    bv = np.asarray(bv, np.float32)
    bo = np.asarray(bo, np.float32)
    assert hs.shape == (BATCH, SEQ, EMBED_DIM)
    # biases bq/bk/bv are zero in this problem; fold nonzero ones on host
    # by shifting is impossible (they pass through nonlinearities), so
    # guard loudly rather than silently returning wrong results.
    for name, bias in (("bq", bq), ("bk", bk), ("bv", bv)):
        if np.abs(bias).max() != 0:
            raise NotImplementedError(f"nonzero {name} not supported")

    from concourse.bass_utils import run_bass_kernel_spmd

    if _NC_CACHE is None:
        _NC_CACHE = _trace_core_program()
    nc = _NC_CACHE

    in_maps = make_in_maps(hs, wq, wk, wv, wo)
    res = run_bass_kernel_spmd(nc, in_maps, list(range(N_CORES)))
    acc = np.zeros((EMBED_DIM, T), np.float32)
    for c in range(N_CORES):
        acc += res.results[c]["out"].astype(np.float32)
    out = acc.T + bo[None, :]
    return out.reshape(BATCH, SEQ, EMBED_DIM).astype(np.float32)


# revision 46
# speedup vs baseline: 1.0791x; 1.0791x over previous
"""BigBird block-sparse attention for Trainium2, 8-core SPMD.

Sharding: head-parallel. Each core owns 2 of the 16 heads (both batches).
  - q/k/v projections computed only for the core's 128 feature slice
    (full hidden_states replicated, weights sliced column-wise).
  - attention fully local per (batch, head).
  - out_proj tensor-parallel on the head (contraction) dim: each core
    emits a full-shape partial; the host sums the 8 partials and adds
    the output bias.

Performance model notes (measured on HW): the PE is INSTRUCTION-ISSUE
bound at ~110ns per matmul when matmuls are narrow — wide matmuls are
essential.  The attention schedule therefore uses UNION runs: for each
128-key tile, one full-height score matmul per run of consecutive
query blocks attending EITHER half (64-key block) of the tile.  Query
blocks attending only one half get the other half's exp values zeroed
in SBUF (cheap DVE/Pool memsets) so that a single K=128 AV matmul per
run is correct — the appended ones column then also yields the correct
softmax denominator Z.

On-device layout choices:
  - activations feature-major (features on partitions, tokens on free dim)
  - scores computed transposed: S_T[key, query] = k_j^T q, so that
    * AV is a natural matmul (contraction = keys = partitions),
    * the softmax denominator Z falls out of a ones-column appended to V^T,
    * normalization folds into the PSUM->SBUF context copy as a
      partition-broadcast multiply by 1/Z.
  - softmax skips max-subtraction (scores are O(1) after the 1/8 scale;
    exp cannot overflow fp32 for this distribution; softmax is shift
    invariant so the reference is matched).
  - v is projected feature-major on all 128 partitions (both heads) and
    transposed per 128-token tile with ONE PE transpose, then copied to
    the [v | 1] slots (vaug2) both heads at once.
  - out_proj for batch 0 is interleaved into the attention phase so the
    finalize chain of the last pair is hidden behind real PE work.
"""

import numpy as np
import ml_dtypes
from contextlib import ExitStack

# ----- problem constants (hardcoded per contract) --------------------------
EMBED_DIM = 1024
NUM_HEADS = 16
HEAD_DIM = 64           # d per head
WINDOW = 3
N_RAND = 3
BLOCK = 64
BATCH = 2
SEQ = 2048
NB = SEQ // BLOCK       # 32 key/query blocks per sequence
N_CORES = 8
HPC = NUM_HEADS // N_CORES      # heads per core = 2
FPC = HPC * HEAD_DIM            # feature slice per core = 128
T = BATCH * SEQ                 # 4096 tokens
NKT = NB // 2                   # 16 key tiles of 128 keys per (b,h)
SCALE = HEAD_DIM ** -0.5

BF16 = ml_dtypes.bfloat16
FP8 = ml_dtypes.float8_e4m3fn

# score-chunk window width in psum columns (2 PSUM banks; 3-deep ring so
# the PE can run chunks ahead of the exp stream)
CHUNK_W = 1024
PSUM_BANK = 512  # fp32 elements per bank


def _block_attend() -> np.ndarray:
    """attend[r, kb]: query block r attends key block kb.

    Block-granular replica of the reference _bigbird_mask (the mask is
    block-constant: global first block rows/cols, +-WINDOW band, and
    N_RAND random blocks per row drawn with RandomState(0))."""
    att = np.zeros((NB, NB), dtype=bool)
    att[0, :] = True
    att[:, 0] = True
    blk = np.arange(NB)
    att |= np.abs(blk[:, None] - blk[None, :]) <= WINDOW
    rng = np.random.RandomState(0)
    for b in range(1, NB):
        avail = [x for x in range(1, NB) if abs(x - b) > WINDOW]
        if avail:
            sel = rng.choice(avail, size=min(N_RAND, len(avail)), replace=False)
            att[b, sel] = True
    return att


def _runs_of(mask_1d: np.ndarray):
    """[(r0, nblocks)] maximal runs of consecutive True entries."""
    runs = []
    for r in np.flatnonzero(mask_1d):
        if runs and runs[-1][0] + runs[-1][1] == r:
            runs[-1][1] += 1
        else:
            runs.append([int(r), 1])
    return [(r0, n) for r0, n in runs]


def build_schedule(qb_lo=0, qb_hi=NB):
    """Union-run schedule, packed into CHUNK_W-wide psum windows.

    Returns list of chunks; each chunk is a dict:
      W     : used width in psum columns
      runs  : [(j, r0, nblk, off)] score/AV runs — query blocks
              r0..r0+nblk attend at least one half of key tile j; the
              score matmul is full-height (128 keys), AV is one K=128
              matmul per run against the [v|1] slot of tile j.
      zeros : [(row0, nrows, off, w)] exp outputs to zero in E (the
              unattended half of single-side columns).
    """
    att = _block_attend()
    chunks = []
    cur = dict(W=0, runs=[], zeros=[])
    for j in range(NKT):
        a0, a1 = att[:, 2 * j], att[:, 2 * j + 1]
        u = (a0 | a1).copy()
        # merge runs separated by exactly 1 block (measured optimum:
        # per-matmul overhead beats the extra 64 exp/AV columns at gap 1
        # but not at gap 2); gap columns get fully zeroed in E (harmless
        # for AV and Z)
        u[:qb_lo] = False
        u[qb_hi:] = False
        g0 = None
        for r0, n in _runs_of(u):
            if g0 is not None and r0 - g0 <= 1:
                u[g0:r0] = True
            g0 = r0 + n
        for r0, n in _runs_of(u):
            while n > 0:
                space = (CHUNK_W - cur["W"]) // 64
                if space == 0:
                    chunks.append(cur)
                    cur = dict(W=0, runs=[], zeros=[])
                    continue
                take = min(n, space)
                off = cur["W"]
                cur["runs"].append((j, r0, take, off))
                # zero rects for single-side query blocks, merged along
                # consecutive same-type blocks
                qq = r0
                while qq < r0 + take:
                    def typ(q):
                        if a0[q] and a1[q]:
                            return 2
                        if a0[q]:
                            return 0
                        if a1[q]:
                            return 1
                        return 3  # merged gap: zero all 128 rows
                    t = typ(qq)
                    q2 = qq
                    while q2 < r0 + take and typ(q2) == t:
                        q2 += 1
                    if t == 3:
                        cur["zeros"].append((0, 128,
                                             off + 64 * (qq - r0),
                                             64 * (q2 - qq)))
                    elif t != 2:
                        # even-only (t=0): zero odd rows 64:128;
                        # odd-only (t=1): zero even rows 0:64
                        cur["zeros"].append((64 * (1 - t), 64,
                                             off + 64 * (qq - r0),
                                             64 * (q2 - qq)))
                    qq = q2
                cur["W"] += 64 * take
                r0 += take
                n -= take
    if cur["runs"]:
        chunks.append(cur)
    for c in chunks:
        zs = sorted(c["zeros"])
        merged = []
        for z in zs:
            if merged and merged[-1][0] == z[0] and \
                    merged[-1][1] == z[1] and \
                    merged[-1][2] + merged[-1][3] == z[2]:
                merged[-1] = (merged[-1][0], merged[-1][1], merged[-1][2],
                              merged[-1][3] + z[3])
            else:
                merged.append(z)
        c["zeros"] = merged
    return chunks


def _bank_split(off, w, bank=PSUM_BANK):
    """split [off, off+w) at bank boundaries -> [(off, w), ...]"""
    out = []
    while w > 0:
        room = bank - (off % bank)
        take = min(room, w)
        out.append((off, take))
        off += take
        w -= take
    return out


# ---------------------------------------------------------------------------
# numpy golden of the exact on-device algorithm (fp32, validates schedule)
# ---------------------------------------------------------------------------
def numpy_golden(hidden_states, wq, bq, wk, bk, wv, bv, wo, bo):
    hs = np.asarray(hidden_states, np.float32).reshape(T, EMBED_DIM)
    chunks = (build_schedule(0, NB // 2) + build_schedule(NB // 2, NB))
    out = np.zeros((T, EMBED_DIM), np.float32)
    for c in range(N_CORES):
        f = slice(FPC * c, FPC * (c + 1))
        q = hs @ np.asarray(wq, np.float32)[f, :].T  # (T, 128)
        k = hs @ np.asarray(wk, np.float32)[f, :].T
        v = hs @ np.asarray(wv, np.float32)[f, :].T
        ctx_all = np.zeros((FPC, T), np.float32)
        for b in range(BATCH):
            for hl in range(HPC):
                d = slice(64 * hl, 64 * hl + 64)
                tok = slice(b * SEQ, (b + 1) * SEQ)
                qb = q[tok, d]   # (2048, 64)
                kb = k[tok, d]
                vb = v[tok, d]
                v_aug = np.concatenate(
                    [vb, np.ones((SEQ, 64), np.float32)], 1)
                ctx = np.zeros((128, SEQ), np.float32)
                for ch in chunks:
                    E = np.zeros((128, ch["W"]), np.float32)
                    for j, r0, nblk, off in ch["runs"]:
                        kk = slice(j * 128, j * 128 + 128)
                        qq = slice(64 * r0, 64 * (r0 + nblk))
                        s = kb[kk, :] @ qb[qq, :].T  # (128 keys, queries)
                        E[:, off:off + 64 * nblk] = np.exp(SCALE * s)
                    for row0, nrows, off, w in ch["zeros"]:
                        E[row0:row0 + nrows, off:off + w] = 0.0
                    for j, r0, nblk, off in ch["runs"]:
                        kk = slice(j * 128, j * 128 + 128)
                        qq = slice(64 * r0, 64 * (r0 + nblk))
                        ctx[:, qq] += v_aug[kk, :].T @ E[:, off:off + 64 * nblk]
                ctx_n = ctx[:64, :] / ctx[64:65, :]
                ctx_all[d, tok] = ctx_n
        partial = np.asarray(wo, np.float32)[:, f] @ ctx_all  # (1024, T)
        out += partial.T
    out = out + np.asarray(bo, np.float32)
    return out.reshape(BATCH, SEQ, EMBED_DIM)


# ---------------------------------------------------------------------------
# Bass/Tile kernel (one core's program; SPMD across 8 cores)
# ---------------------------------------------------------------------------
def _trace_core_program():
    import concourse.bass as bass
    import concourse.mybir as mybir
    import concourse.tile as tile
    from concourse import bacc

    dt = mybir.dt
    chunks_h = [build_schedule(0, NB // 2), build_schedule(NB // 2, NB)]

    nc = bacc.Bacc(None, target_bir_lowering=False)
    with tile.TileContext(nc) as tc:
        with ExitStack() as top:
            dram = top.enter_context(tc.tile_pool(name="dram", bufs=1, space="DRAM"))
            hT_d = dram.tile([EMBED_DIM, T], dt.bfloat16, kind="ExternalInput",
                             name="hT", uniquify=False)
            wqkvT_d = dram.tile([EMBED_DIM, 3 * FPC], dt.bfloat16,
                                kind="ExternalInput", name="wqkvT",
                                uniquify=False)
            woT_d = dram.tile([FPC, EMBED_DIM], dt.bfloat16,
                              kind="ExternalInput", name="woT", uniquify=False)
            ident_d = dram.tile([128, 128], dt.bfloat16,
                                kind="ExternalInput", name="ident",
                                uniquify=False)
            out_d = dram.tile([EMBED_DIM, T], dt.bfloat16,
                              kind="ExternalOutput", name="out", uniquify=False)

            # ---- persistent SBUF tensors -----------------------------------
            persist = top.enter_context(tc.tile_pool(name="persist", bufs=1))
            # NOTE: fp8e4m3 projection operands with DoubleRow matmuls were
            # tried (works, ~10us faster) but the quantization error of a
            # random-sign dot product does NOT average down with contraction
            # length — measured 6.5%% relative error, over the 2%% budget.
            wqkv = persist.tile([128, 8, 3 * FPC], dt.bfloat16, name="wqkv_sb")
            woT = persist.tile([128, EMBED_DIM], dt.bfloat16, name="wo_sb")
            ident = persist.tile([128, 128], dt.bfloat16, name="ident_sb")
            # q/k head-major on 64 partitions (base-0 only: matmuls with
            # base-partition-64 contraction operands hit a codegen/HW bug)
            q_sb = persist.tile([64, HPC * T], dt.bfloat16, name="q_sb")
            k_sb = persist.tile([64, HPC * T], dt.bfloat16, name="k_sb")
            # v feature-major, both heads on 128 partitions (transposed
            # per-tile on the PE)
            vfm = persist.tile([128, T], dt.bfloat16, name="vfm_sb")
            # per (b,hl): [v | 1...1] slots per 128-key tile, K=128 for
            # all AV.  Slot cols 64:128 are ALL ones: every AV matmul then
            # emits Z replicated across output rows 64:128 — the partition
            # broadcast of the softmax denominator comes free on the PE
            # (matmul cost depends only on the moving columns).
            vaug2 = persist.tile([128, BATCH * HPC, NKT * 128], dt.bfloat16,
                                 name="vaug2_sb")
            # normalized context, one tile per batch so out-proj for batch 0
            # doesn't pick up a false dependency on batch-1 writers
            ctx_b = [persist.tile([128, SEQ], dt.bfloat16, name=f"ctx_sb{b}")
                     for b in range(BATCH)]
            zt = persist.tile([128, 256], dt.bfloat16, name="zt_sb")

            # weight DMAs first, one per contraction chunk (finer deps:
            # the e=0 projection matmul starts as soon as slice 0 lands)
            wq_view = wqkvT_d.rearrange("(e p) f -> p e f", p=128)
            for e in range(8):
                (nc.gpsimd if e % 2 else nc.scalar).dma_start(
                    out=wqkv[:, e, :], in_=wq_view[:, e, :])

            NCHUNK = T // 512
            hT_pool = tc.tile_pool(name="hT_pool", bufs=1)
            with hT_pool as hp:
                hT = hp.tile([128, 8, T], dt.bfloat16, name="hT_sb")

                # hT DMA dispatches spread over 3 engine queues (descriptor
                # generation is ~600ns of engine time apiece); all issued
                # before any memset/copy work so the queues drain into DMA
                # as early as possible
                def h_dma(n, e):
                    eng = (nc.sync if e < 4 else
                           nc.scalar if e < 6 else nc.gpsimd)
                    eng.dma_start(
                        out=hT[:, e, 512 * n:512 * n + 512],
                        in_=hT_d[128 * e:128 * e + 128, 512 * n:512 * n + 512])

                for e in range(8):
                    h_dma(0, e)
                # wo/ident aren't needed until attention — dispatch after
                # the first hidden-state chunk so they don't delay it
                nc.gpsimd.dma_start(out=woT[:], in_=woT_d[:])
                nc.gpsimd.dma_start(out=ident[:], in_=ident_d[:])

                # ~3us of dummy matmuls while DMAs stream: keeps the PE
                # pipeline warm so the projection starts at full p-state,
                # and writes every cell of all 8 PSUM banks once (launders
                # boot-garbage psum).  Closed before proj pools open.
                nc.vector.memset(zt[:], 0.0)
                with tc.tile_pool(name="warm_ps", bufs=8, space="PSUM") as wps:
                    for i in range(10):
                        w = wps.tile([128, 512], dt.float32, tag="warm")
                        nc.tensor.matmul(w[:, 0:256], zt[:, 0:128],
                                         zt[:, 0:256], start=True, stop=True,
                                         skip_group_check=True)

                for n in range(1, NCHUNK):
                    for e in range(8):
                        h_dma(n, e)

                # vaug2 ones-half preset (strided memsets on DVE)
                for p in range(BATCH * HPC):
                    s2 = vaug2[:, p, :].rearrange("p (m c) -> p m c", c=128)
                    nc.vector.memset(s2[:, :, 64:128], 1.0)

                # ---- phase 1: q/k/v projections + per-tile v transpose -----
                # psum->sbuf copies: only DVE and ACT can read PSUM.
                # Projection copies go on DVE alone — the ACT queue holds
                # ~15us of hT DMA dispatches at this point and would stall
                # the proj psum ring.  The vaug2 slot copies (not needed
                # until attention) alternate DVE/ACT.
                cp = nc.vector.tensor_copy
                vt_rot = [nc.vector.tensor_copy, nc.scalar.copy]
                vt_i = [0]

                def vt_cp(dst, src):
                    vt_rot[vt_i[0] % 2](dst, src)
                    vt_i[0] += 1

                with tc.tile_pool(name="proj_ps", bufs=2, space="PSUM") as pps, \
                        tc.tile_pool(name="vt_ps", bufs=2, space="PSUM") as vtp:
                    for n in range(NCHUNK):
                        tsl = slice(512 * n, 512 * (n + 1))
                        # q and k (feature-major, head-split into q_sb/k_sb)
                        for tg, wsl, dst in [("pq", slice(0, 128), q_sb),
                                             ("pk", slice(128, 256), k_sb)]:
                            ps = pps.tile([128, 512], dt.float32, tag=tg)
                            for e in range(8):
                                nc.tensor.matmul(ps[:], wqkv[:, e, wsl],
                                                 hT[:, e, tsl],
                                                 start=(e == 0), stop=(e == 7))
                            for hl in range(HPC):
                                cp(dst[:, hl * T + 512 * n:
                                       hl * T + 512 * n + 512],
                                   ps[64 * hl:64 * hl + 64, :])
                        # v: both heads on 128 partitions, single wide copy
                        ps = pps.tile([128, 512], dt.float32, tag="pv")
                        for e in range(8):
                            nc.tensor.matmul(ps[:], wqkv[:, e, 256:384],
                                             hT[:, e, tsl],
                                             start=(e == 0), stop=(e == 7))
                        cp(vfm[:, tsl], ps[:])
                        # transpose the 4 completed 128-token tiles of the
                        # PREVIOUS chunk (vfm write must land first; lag one
                        # chunk so the copy has completed)
                        for gg in range(4):
                            g = 4 * (n - 1) + gg if n > 0 else None
                            if g is None:
                                break
                            b, jj = g // NKT, g % NKT
                            vt = vtp.tile([128, 128], dt.bfloat16, tag="vt")
                            nc.tensor.transpose(
                                vt[:], vfm[:, 128 * g:128 * g + 128],
                                ident[:])
                            p0 = b * HPC
                            vt_cp(vaug2[:, p0:p0 + 2, 128 * jj:128 * jj + 64],
                                  vt[:].rearrange("p (h c) -> p h c", h=2))
                    # last chunk's 4 tiles
                    for gg in range(4):
                        g = 4 * (NCHUNK - 1) + gg
                        b, jj = g // NKT, g % NKT
                        vt = vtp.tile([128, 128], dt.bfloat16, tag="vt")
                        nc.tensor.transpose(
                            vt[:], vfm[:, 128 * g:128 * g + 128], ident[:])
                        p0 = b * HPC
                        vt_cp(vaug2[:, p0:p0 + 2, 128 * jj:128 * jj + 64],
                              vt[:].rearrange("p (h c) -> p h c", h=2))

            # ---- attention + interleaved out-projection --------------------
            def emit_pair(b, hl, scp, ctxp, ep, fp, tag):
                p = b * HPC + hl
                qtok0 = hl * T + b * SEQ  # column base in q/k (head-major)
                HSEQ = SEQ // 2
                for half in range(2):
                    qoff = half * HSEQ  # query-column base of this half
                    ctx = ctxp.tile([128, HSEQ], dt.float32, tag="ctx")
                    # PSUM start=True arms the whole bank for lazy zeroing:
                    # issue it exactly once per ctx bank (the j=0 runs
                    # cover every query column, so all banks start early).
                    ctx_bank_started = [False] * (HSEQ // PSUM_BANK)

                    E_tiles = []
                    zi = 0
                    chunks = chunks_h[half]
                    for ci, ch in enumerate(chunks):
                        W = ch["W"]
                        S = scp.tile([128, CHUNK_W], dt.float32, tag="S")
                        E = ep.tile([128, W], dt.bfloat16,
                                    tag=f"E{tag}{half}{ci}",
                                    name=f"E{tag}{half}{ci}")
                        E_tiles.append(E)
                        for j, r0, nblk, off in ch["runs"]:
                            kcol0 = qtok0 + 128 * j
                            for o, w in _bank_split(off, 64 * nblk):
                                qc = qtok0 + 64 * r0 + (o - off)
                                nc.tensor.matmul(
                                    S[:, o:o + w],
                                    k_sb[:, kcol0:kcol0 + 128],
                                    q_sb[:, qc:qc + w],
                                    start=True, stop=True)
                        nc.scalar.activation(
                            E[:, :W], S[:, :W],
                            mybir.ActivationFunctionType.Exp, scale=SCALE)
                        # zero the unattended half of single-side columns
                        for row0, nrows, off, w in ch["zeros"]:
                            (nc.vector if zi % 2 else nc.gpsimd).memset(
                                E[row0:row0 + nrows, off:off + w], 0.0)
                            zi += 1
                    # AV accumulate (+Z via ones half), K=128 always
                    for ci, ch in enumerate(chunks):
                        E = E_tiles[ci]
                        for j, r0, nblk, off in ch["runs"]:
                            lhsT = vaug2[:, p, 128 * j:128 * j + 128]
                            for o, w in _bank_split(64 * r0, 64 * nblk):
                                eo = off + (o - 64 * r0)
                                co = o - qoff
                                bank = co // PSUM_BANK
                                st = not ctx_bank_started[bank]
                                ctx_bank_started[bank] = True
                                nc.tensor.matmul(
                                    ctx[:, co:co + w], lhsT,
                                    E[:, eo:eo + w],
                                    start=st, stop=False,
                                    skip_group_check=True)
                    # finalize: ctx rows 64:128 already hold Z on every
                    # row (ones half of the vaug2 slots), so 1/Z is a
                    # straight 64-partition reciprocal — no DMA spread, no
                    # partition broadcast.  reciprocal_approx_fast is one
                    # DVE instruction (~51 ULP, ~5x faster than the
                    # bit-exact iterative divide).
                    zsb = fp.tile([64, HSEQ], dt.float32, tag="zsb")
                    nc.vector.tensor_copy(zsb[:], ctx[64:128, :])
                    rbc = fp.tile([64, HSEQ], dt.float32, tag="rbc")
                    nc.vector.reciprocal_approx_fast(rbc[:], zsb[:])
                    for cc in range(HSEQ // 512):
                        csl = slice(512 * cc, 512 * (cc + 1))
                        nc.vector.tensor_tensor(
                            out=ctx_b[b][64 * hl:64 * hl + 64,
                                         qoff + 512 * cc:
                                         qoff + 512 * cc + 512],
                            in0=ctx[0:64, csl],
                            in1=rbc[:, csl],
                            op=mybir.AluOpType.mult)

            def emit_oproj(opp, opsb, eo_list, b, engines):
                # 2-bank psum tiles: 2 matmuls share one wide copy (fewer
                # psum->sbuf instructions; only DVE/ACT can read PSUM)
                ei = 0
                for eo in eo_list:
                    ob = opsb.tile([128, SEQ], dt.bfloat16, tag="ob")
                    for nn in range(SEQ // 1024):
                        ps = opp.tile([128, 1024], dt.float32, tag="op")
                        for half in range(2):
                            csl = slice(1024 * nn + 512 * half,
                                        1024 * nn + 512 * half + 512)
                            nc.tensor.matmul(
                                ps[:, 512 * half:512 * half + 512],
                                woT[:, 128 * eo:128 * eo + 128],
                                ctx_b[b][:, csl], start=True, stop=True)
                        engines[ei % len(engines)](
                            ob[:, 1024 * nn:1024 * nn + 1024], ps[:])
                        ei += 1
                    (nc.sync if eo % 2 else nc.gpsimd).dma_start(
                        out=out_d[128 * eo:128 * eo + 128,
                                  b * SEQ:(b + 1) * SEQ],
                        in_=ob[:])

            # segment 1: pairs (0,0),(0,1),(1,0) — scp before ctxp so the
            # out-proj pool that follows lands on the score banks (whose
            # last readers finished long ago), not the ctx banks
            with tc.tile_pool(name="sc_ps", bufs=3, space="PSUM") as scp, \
                    tc.tile_pool(name="ctx_ps", bufs=1, space="PSUM") as ctxp, \
                    tc.tile_pool(name="e_pool", bufs=1) as ep, \
                    tc.tile_pool(name="fin_pool", bufs=3) as fp:
                emit_pair(0, 0, scp, ctxp, ep, fp, "a")
                emit_pair(0, 1, scp, ctxp, ep, fp, "a")
                emit_pair(1, 0, scp, ctxp, ep, fp, "a")

            # out-proj batch 0, first half of eo (runs while pair (1,0)'s
            # finalize chain completes; copies on ACT so the DVE-resident
            # finalize chain doesn't block them)
            with tc.tile_pool(name="op_ps1", bufs=2, space="PSUM") as opp, \
                    tc.tile_pool(name="op_sb1", bufs=2) as opsb:
                emit_oproj(opp, opsb, range(0, 4), 0,
                           [nc.vector.tensor_copy, nc.scalar.copy,
                            nc.vector.tensor_copy, nc.vector.tensor_copy])

            # segment 2: last pair (1,1)
            with tc.tile_pool(name="sc_ps2", bufs=3, space="PSUM") as scp, \
                    tc.tile_pool(name="ctx_ps2", bufs=1, space="PSUM") as ctxp, \
                    tc.tile_pool(name="e_pool2", bufs=1) as ep, \
                    tc.tile_pool(name="fin_pool2", bufs=1) as fp:
                emit_pair(1, 1, scp, ctxp, ep, fp, "b")

            # out-proj: rest of batch 0 (covers the last finalize chain,
            # ACT copies), then batch 1 (alternating engines)
            with tc.tile_pool(name="op_ps2", bufs=3, space="PSUM") as opp, \
                    tc.tile_pool(name="op_sb2", bufs=3) as opsb:
                emit_oproj(opp, opsb, range(4, 8), 0, [nc.scalar.copy])
                emit_oproj(opp, opsb, range(0, 8), 1,
                           [nc.scalar.copy, nc.vector.tensor_copy])

    nc.compile()
    _dedup_ldweights(nc)
    return nc


def _dedup_ldweights(nc):
    """Remove PE weight reloads of the already-loaded stationary tile.

    bass pairs every matmul with an explicit InstLdweights; on HW the
    load serializes ~130ns of PE time apiece.  Consecutive matmuls here
    frequently share the stationary operand (score runs per key tile,
    AV bank splits per [v|1] slot, out-proj column tiles per eo slice),
    so the repeat loads are pure waste.  Only waitless/updateless loads
    are removed, which leaves every semaphore count intact."""
    removed = 0
    for blk in nc.main_func.blocks:
        insts = blk.instructions
        last_key = None
        to_remove = []
        for i in insts:
            eng = getattr(i, "engine", None)
            if eng is None or str(eng) != "EngineType.PE":
                continue
            tn = type(i).__name__
            if tn == "InstLdweights":
                key = (str(i.ins[0]), str(getattr(i, "perf_mode", None)),
                       str(getattr(i, "is_transpose", None)),
                       str(getattr(i, "tile_position", None)))
                si = i.sync_info
                clean = si is None or (len(si.on_wait) == 0
                                       and len(si.on_update) == 0)
                if clean and key == last_key:
                    to_remove.append(i)
                else:
                    last_key = key
            elif tn in ("InstMatmult", "InstEventSemaphore"):
                pass  # neither clobbers the loaded weights
            else:
                last_key = None
        for i in to_remove:
            insts.remove(i)
        removed += len(to_remove)


_NC_CACHE = None


def make_in_maps(hs, wq, wk, wv, wo):
    hT = np.ascontiguousarray(
        np.asarray(hs, np.float32).reshape(T, EMBED_DIM).T).astype(BF16)
    ident = np.eye(128, dtype=np.float32).astype(BF16)
    wq = np.asarray(wq, np.float32)
    wk = np.asarray(wk, np.float32)
    wv = np.asarray(wv, np.float32)
    wo = np.asarray(wo, np.float32)
    in_maps = []
    for c in range(N_CORES):
        f = slice(FPC * c, FPC * (c + 1))
        wqkvT = np.concatenate([wq[f, :].T, wk[f, :].T, wv[f, :].T], axis=1)
        in_maps.append({
            "hT": hT,
            "wqkvT": np.ascontiguousarray(wqkvT).astype(BF16),
            "woT": np.ascontiguousarray(wo[:, f].T).astype(BF16),
            "ident": ident,
        })
    return in_maps


def kernel(hidden_states, wq, bq, wk, bk, wv, bv, wo, bo):
    global _NC_CACHE
    hs = np.asarray(hidden_states, np.float32)
    wq = np.asarray(wq, np.float32)
    wk = np.asarray(wk, np.float32)
    wv = np.asarray(wv, np.float32)
    wo = np.asarray(wo, np.float32)
    bq = np.asarray(bq, np.float32)
    bk = np.asarray(bk, np.float32)
    bv = np.asarray(bv, np.float32)
    bo = np.asarray(bo, np.float32)
    assert hs.shape == (BATCH, SEQ, EMBED_DIM)
    # biases bq/bk/bv are zero in this problem; fold nonzero ones on host
    # by shifting is impossible (they pass through nonlinearities), so
    # guard loudly rather than silently returning wrong results.
    for name, bias in (("bq", bq), ("bk", bk), ("bv", bv)):
        if np.abs(bias).max() != 0:
            raise NotImplementedError(f"nonzero {name} not supported")

    from concourse.bass_utils import run_bass_kernel_spmd

    if _NC_CACHE is None:
        _NC_CACHE = _trace_core_program()
    nc = _NC_CACHE

    in_maps = make_in_maps(hs, wq, wk, wv, wo)
    res = run_bass_kernel_spmd(nc, in_maps, list(range(N_CORES)))
    acc = np.zeros((EMBED_DIM, T), np.float32)
    for c in range(N_CORES):
        acc += res.results[c]["out"].astype(np.float32)
    out = acc.T + bo[None, :]
    return out.reshape(BATCH, SEQ, EMBED_DIM).astype(np.float32)
